# revision 53
# baseline (speedup 1.0000x reference)
"""Fused 2-layer KAN for Trainium2, data-parallel across 8 NeuronCores.

Math: with G=3 grid points the spline basis is piecewise-linear in x, so each
KAN layer collapses to a small dense matmul over 3 cheap feature maps:

    out = bias + silu(x) @ Wb + u @ P1 + C @ (P2 - P1)
      u = clip(x, -1, 1),  C = max(u, 0)
      Wb = imp*bw;  T = imp*sw*cp;  P1 = T@(bv1-bv0);  P2 = T@(bv2-bv1)
      bias_j = sum_i T[i,j,:] @ bv1

All K=5 spline control points fold into P1/P2/bias on the host (O(I*J*K) work).

Layout: rows are assigned block-contiguously (partition p holds rows 8p+j
of each 1024-row macro) so both HBM DMAs are 2KB-contiguous per partition
(the baseline's 256B-line gathers cost ~1.1us of SWDGE issue per macro).
The steady state is ACT/DVE-throughput-bound (~2.5/2.6us per macro), so h
lives in one 2-bank PSUM tile and silu(h) is a single merged ACT op.  That
merge creates a serialized cross-engine chain u2B -> sl2 -> u2A (Tile
serializes same-PSUM-bank access pairs), so the next macro's u1/c1 are
issued between the two u2 halves: the DVE fills the sl2 window instead of
idling.  A 3-stage modulo software pipeline (transpose/L1 of macro m+1
issued inside macro m) keeps the in-order engine queues from
head-blocking, and the PE warmup burst is split around the first
transpose so it fills the maps1(0) wait instead of delaying it.
"""

import os
import sys
from contextlib import ExitStack

import numpy as np
import ml_dtypes

for _p in ("/opt/trn_rl_repo",):
    if _p not in sys.path and os.path.isdir(_p):
        sys.path.insert(0, _p)

import concourse.bass as bass
import concourse.tile as tile
from concourse import bacc, mybir
from concourse.bass_utils import run_bass_kernel_spmd
from concourse.masks import make_identity

F32 = mybir.dt.float32
BF16 = mybir.dt.bfloat16
BF = ml_dtypes.bfloat16

N_CORES = 8
D0, D1, D2 = 64, 128, 64
K, DEG, G, LO, HI = 5, 3, 3, -1.0, 1.0
MACRO = 1024  # batch rows per device macro-iteration

_nc_cache = {}


def _basis_table():
    knots = np.linspace(LO - DEG * 0.1, HI + DEG * 0.1, K + DEG + 1)
    grid = np.linspace(LO, HI, G)
    bv = np.zeros((G, K), dtype=np.float32)
    for i in range(K):
        center = (knots[i + DEG // 2] + knots[i + DEG // 2 + 1]) / 2.0
        width = (knots[i + DEG + 1] - knots[i]) / 2.0
        bv[:, i] = np.exp(-(((grid - center) / width) ** 2))
    bv = bv / (bv.sum(axis=1, keepdims=True) + 1e-6)
    return bv


def _prep_consts(cp0, bw0, sw0, imp0, cp1, bw1, sw1, imp1):
    f8 = np.float64
    bv = _basis_table().astype(f8)
    d1, d2 = bv[1] - bv[0], bv[2] - bv[1]

    def fold(cp, bw, sw, imp):
        T = imp.astype(f8)[:, :, None] * sw.astype(f8)[:, :, None] * cp.astype(f8)
        Wb = imp.astype(f8) * bw.astype(f8)
        return Wb, T @ d1, T @ d2, (T @ bv[1]).sum(axis=0)

    Wb0, P10, P20, b1 = fold(cp0, bw0, sw0, imp0)
    Wb1, P11, P21, b2 = fold(cp1, bw1, sw1, imp1)
    bias2_eff = b2 + b1 @ P21

    w1 = np.stack([Wb0, P10, P20 - P10], axis=0)  # [3, 64, 128] lhsT chunks
    w1 = np.concatenate([w1, w1], axis=1)  # duplicate rows for partitions 64-127
    w1 = np.ascontiguousarray(w1.transpose(1, 0, 2)).reshape(128, 384)
    w2 = np.stack([Wb1, P11, P21 - P11], axis=0)  # [3, 128, 64] rhs chunks
    w2 = np.ascontiguousarray(w2.transpose(1, 0, 2)).reshape(128, 192)

    return {
        "wpk": np.concatenate([w1, w2], axis=1).astype(BF),  # [128, 576]
        "spk": np.stack(
            [b1, -1.0 - b1, 1.0 - b1, -b1, np.zeros_like(b1)], axis=1
        ).astype(np.float32),  # [128, 5] = b1|s1|s2|nb1|zero
        "b2row": np.tile(bias2_eff, 8).astype(BF).reshape(1, 512),
        "identc": np.eye(128, dtype=np.float32).astype(BF),
        "onesc": np.ones((1, 128), dtype=np.float32).astype(BF),
    }


def _build(rows):
    assert rows % MACRO == 0
    nc = bacc.Bacc(
        "TRN2",
        target_bir_lowering=False,
        debug=False,
        enable_asserts=False,
        num_devices=N_CORES,
    )
    xd = nc.dram_tensor("x", [rows, D0], F32, kind="ExternalInput")
    wpkd = nc.dram_tensor("wpk", [128, 576], BF16, kind="ExternalInput")
    spkd = nc.dram_tensor("spk", [128, 5], F32, kind="ExternalInput")
    b2d = nc.dram_tensor("b2row", [1, 512], BF16, kind="ExternalInput")
    identd = nc.dram_tensor("identc", [128, 128], BF16, kind="ExternalInput")
    onesd = nc.dram_tensor("onesc", [1, 128], BF16, kind="ExternalInput")
    outd = nc.dram_tensor("out", [rows, D2], F32, kind="ExternalOutput")

    n_macro = rows // MACRO
    MAX, MIN = mybir.AluOpType.max, mybir.AluOpType.min
    SILU = mybir.ActivationFunctionType.Silu

    with tile.TileContext(nc) as tc, ExitStack() as ctx:
        consts = ctx.enter_context(tc.tile_pool(name="consts", bufs=1))
        xin = ctx.enter_context(tc.tile_pool(name="xin", bufs=4))
        f1 = ctx.enter_context(tc.tile_pool(name="f1", bufs=3))
        f2 = ctx.enter_context(tc.tile_pool(name="f2", bufs=3))
        osb = ctx.enter_context(tc.tile_pool(name="osb", bufs=3))
        ps_x = ctx.enter_context(tc.tile_pool(name="ps_x", bufs=2, space="PSUM"))
        ps_h = ctx.enter_context(tc.tile_pool(name="ps_h", bufs=2, space="PSUM"))
        ps_o = ctx.enter_context(tc.tile_pool(name="ps_o", bufs=2, space="PSUM"))

        # ident/ones come in as DRAM constants: no gpsimd iota / DVE memset in
        # the preamble, so the input DMAs and PE warmup start sooner.
        ident = consts.tile([128, 128], BF16)
        nc.sync.dma_start(ident, identd.ap())
        ones = consts.tile([1, 128], BF16)
        nc.sync.dma_start(ones, onesd.ap())
        wpk = consts.tile([128, 576], BF16)
        nc.sync.dma_start(wpk, wpkd.ap())
        spk = consts.tile([128, 5], F32)
        nc.sync.dma_start(spk, spkd.ap())
        b2r = consts.tile([1, 512], BF16)
        nc.sync.dma_start(b2r, b2d.ap())
        b1, s1, s2, nb1, zc = (spk[:, i : i + 1] for i in range(5))
        w1c = [wpk[:, c * 128 : (c + 1) * 128] for c in range(3)]
        w2c = [wpk[:, 384 + c * 64 : 384 + (c + 1) * 64] for c in range(3)]

        # Preload the ACT Silu table during the preamble so the first real
        # activation doesn't stall 1.3us on a lazy ACT_TABLE_LOAD.
        slwarm = consts.tile([1, 128], BF16)
        nc.scalar.activation(slwarm, ones, SILU)

        # PE pre-warm: a few dummy matmuls while the input DMA lands (so the
        # HAM clock gate starts opening), the rest issued AFTER the first
        # transpose so they fill the PE's maps1(0) wait window and the clock
        # reaches full speed before the first real L1 matmuls.
        warm = ps_o.tile([128, 8, 64], F32, tag="po")
        for _ in range(4):
            nc.tensor.matmul(warm[:, 0:2], ident, ident, start=True, stop=True)

        def in_dma(m):
            # x[base + 8p + j, f] -> xt[p, j, f]: 2KB contiguous per
            # partition, f32 -> bf16 cast on the SWDGE queue.
            xt = xin.tile([128, 8, 64], BF16, tag="xt")
            src = bass.AP(xd, m * MACRO * 64, [[512, 128], [1, 512]])
            nc.gpsimd.dma_start(xt, src)
            return xt

        def transpose(xt):
            # PE transpose: px[:, q] partitions 0-63 = feats of rows 8p+2q
            # (A-stream), 64-127 = rows 8p+2q+1 (B-stream); free dim = p.
            px = ps_x.tile([128, 4, 128], BF16, tag="px")
            for q in range(4):
                nc.tensor.transpose(px[:, q], xt[:, 2 * q : 2 * q + 2], ident)
            return px

        def maps1(px):
            # L1 feature maps: u1/c1 chained on DVE, silu on ACT
            u1 = f1.tile([128, 4, 128], BF16, tag="u1")
            nc.vector.tensor_scalar(u1, px, -1.0, 1.0, op0=MAX, op1=MIN)
            sl1 = f1.tile([128, 4, 128], BF16, tag="sl1")
            nc.scalar.activation(sl1, px, SILU)
            c1 = f1.tile([128, 4, 128], BF16, tag="c1")
            nc.vector.tensor_scalar_max(c1, u1, zc)
            return u1, sl1, c1

        def l1_mm(mp):
            # L1: two concurrent 64-contraction row-group streams (A=even j
            # on partitions 0-63, B=odd j on 64-127) into one 2-bank tile so
            # sl2 can read all of h with a single merged instruction; chunk
            # order = feature readiness order (u1 -> sl1 -> c1)
            u1, sl1, c1 = mp
            h2 = ps_h.tile([128, 2, 512], F32, tag="h2")
            hA, hB = h2[:, 0], h2[:, 1]
            for i, (c, ft) in enumerate([(1, u1), (0, sl1), (2, c1)]):
                nc.tensor.matmul(hA, w1c[c][0:64], ft[0:64], start=(i == 0), stop=(i == 2))
                nc.tensor.matmul(hB, w1c[c][64:128], ft[64:128], start=(i == 0), stop=(i == 2))
            return h2

        def maps2_first(h2):
            # First half of the L2 feature maps.  The merged sl2 reads BOTH h
            # banks, so Tile serializes it after u2-B (bank B) and before
            # u2-A (bank A): a 2.7us cross-engine chain that would bound the
            # period.  maps1(m+1) is issued between the halves so the DVE
            # fills the sl2 window with u1/c1 instead of idling.
            sl2 = f2.tile([128, 1024], BF16, tag="sl2")
            u2 = f2.tile([128, 1024], BF16, tag="u2")
            nc.vector.tensor_scalar(u2[:, 512:1024], h2[:, 1], s1, s2, op0=MAX, op1=MIN)
            nc.scalar.activation(sl2, h2, SILU, bias=b1)
            return sl2, u2

        def maps2_second(m2, h2):
            sl2, u2 = m2
            c2 = f2.tile([128, 1024], BF16, tag="c2")
            nc.vector.tensor_scalar(u2[:, 0:512], h2[:, 0], s1, s2, op0=MAX, op1=MIN)
            nc.vector.tensor_scalar_max(c2, u2, nb1)
            return sl2, u2, c2

        def l2_mm(sl2, u2, c2):
            # L2 bias init via K=1 ones-matmul (sets has_written on the whole
            # bank so the 24 block matmuls accumulate with start=False).
            po = ps_o.tile([128, 8, 64], F32, tag="po")
            nc.tensor.matmul(po, ones, b2r, start=True, stop=False)
            # Blocks ordered by map readiness: u2's B-half lands first (it
            # is issued before the merged sl2), then sl2, then u2-A, then c2.
            plan = [
                (1, u2, (1, 3, 5, 7)),
                (0, sl2, (0, 2, 4, 6, 1, 3, 5, 7)),
                (1, u2, (0, 2, 4, 6)),
                (2, c2, (0, 1, 2, 3, 4, 5, 6, 7)),
            ]
            for ci, (c, ft, order) in enumerate(plan):
                for gi, g in enumerate(order):
                    off = (g % 2) * 512 + (g // 2) * 128
                    nc.tensor.matmul(
                        po[:, g],
                        ft[:, off : off + 128],
                        w2c[c],
                        start=False,
                        stop=(ci == 3 and gi == 7),
                    )
            return po

        def out_store(m, po):
            # po[p, g, f] = out row 8p+g -> contiguous 2KB per partition
            ot = osb.tile([128, 8, 64], F32, tag="ot")
            nc.scalar.copy(ot, po)
            dst = bass.AP(outd, m * MACRO * 64, [[512, 128], [1, 512]])
            nc.sync.dma_start(dst, ot)

        # 3-stage modulo software pipeline.  Issue order is chosen so each
        # in-order engine queue never head-blocks: PE = [tp(m+1), bias(m),
        # L2(m), L1(m+1)], ACT = [sl2(m), sl1(m+1), ot(m)],
        # DVE = [u2B(m), u1(m+1), c1(m+1), u2A(m), c2(m)].
        xt0 = in_dma(0)
        xt_next = in_dma(1)
        px = transpose(xt0)
        # 4 warmups cover the maps1(0) wait; the rest go AFTER L1(0) where
        # they fill the PE's maps2(0) wait window (~2us) without delaying
        # the first L1, keeping the clock ramping through macro 0.
        for _ in range(4):
            nc.tensor.matmul(warm[:, 0:2], ident, ident, start=True, stop=True)
        h2 = l1_mm(maps1(px))
        for _ in range(12):
            nc.tensor.matmul(warm[:, 0:2], ident, ident, start=True, stop=True)

        for m in range(n_macro):
            m2 = maps2_first(h2)

            # next macro's transpose + L1 maps: the transposes slot ahead of
            # bias/L2 in the PE queue, and u1/c1 fill the DVE while the
            # merged sl2 holds both h banks
            last = m + 1 >= n_macro
            if not last:
                px = transpose(xt_next)
                if m + 2 < n_macro:
                    xt_next = in_dma(m + 2)
                mp = maps1(px)

            f2m = maps2_second(m2, h2)
            po = l2_mm(*f2m)

            # next macro's L1 matmuls, issued before ot(m) so the ACT queue
            # never idles waiting on this macro's L2 matmuls
            if not last:
                h2 = l1_mm(mp)

            out_store(m, po)

    nc.compile()
    return nc


def _get_nc(rows):
    if rows not in _nc_cache:
        _nc_cache[rows] = _build(rows)
    return _nc_cache[rows]


def kernel(x, cp0, bw0, sw0, imp0, cp1, bw1, sw1, imp1, _trace=False, _trace_kwargs=None):
    x = np.ascontiguousarray(np.asarray(x, dtype=np.float32))
    consts = _prep_consts(
        *[np.asarray(a, dtype=np.float32) for a in (cp0, bw0, sw0, imp0, cp1, bw1, sw1, imp1)]
    )
    rows = x.shape[0] // N_CORES
    nc = _get_nc(rows)
    in_maps = []
    for i in range(N_CORES):
        m = dict(consts)
        m["x"] = x[i * rows : (i + 1) * rows]
        in_maps.append(m)
    res = run_bass_kernel_spmd(
        nc, in_maps, list(range(N_CORES)), trace=_trace, **(_trace_kwargs or {})
    )
    out = np.concatenate([res.results[i]["out"] for i in range(N_CORES)], axis=0)
    if _trace:
        return out, res
    return out


# revision 54
# speedup vs baseline: 1.1753x; 1.1753x over previous
"""Fused 2-layer KAN for Trainium2, data-parallel across 8 NeuronCores.

Math: with G=3 grid points the spline basis is piecewise-linear in x, so each
KAN layer collapses to a small dense matmul over 3 cheap feature maps:

    out = bias + silu(x) @ Wb + u @ P1 + C @ (P2 - P1)
      u = clip(x, -1, 1),  C = max(u, 0)
      Wb = imp*bw;  T = imp*sw*cp;  P1 = T@(bv1-bv0);  P2 = T@(bv2-bv1)
      bias_j = sum_i T[i,j,:] @ bv1

All K=5 spline control points fold into P1/P2/bias on the host (O(I*J*K) work).

Layout: rows are assigned block-contiguously (partition p holds rows 8p+j
of each 1024-row macro) so both HBM DMAs are 2KB-contiguous per partition
(the baseline's 256B-line gathers cost ~1.1us of SWDGE issue per macro).
The steady state is ACT/DVE-throughput-bound (~2.5/2.6us per macro), so h
lives in one 2-bank PSUM tile and silu(h) is a single merged ACT op.  That
merge creates a serialized cross-engine chain u2B -> sl2 -> u2A (Tile
serializes same-PSUM-bank access pairs), so the next macro's u1/c1 are
issued between the two u2 halves: the DVE fills the sl2 window instead of
idling.  A 3-stage modulo software pipeline (transpose/L1 of macro m+1
issued inside macro m) keeps the in-order engine queues from
head-blocking, and the PE warmup burst is split around the first
transpose so it fills the maps1(0) wait instead of delaying it.
"""

import os
import sys
from contextlib import ExitStack

import numpy as np
import ml_dtypes

for _p in ("/opt/trn_rl_repo",):
    if _p not in sys.path and os.path.isdir(_p):
        sys.path.insert(0, _p)

import concourse.bass as bass
import concourse.tile as tile
from concourse import bacc, mybir
from concourse.bass_utils import run_bass_kernel_spmd
from concourse.masks import make_identity

F32 = mybir.dt.float32
BF16 = mybir.dt.bfloat16
BF = ml_dtypes.bfloat16

N_CORES = 8
D0, D1, D2 = 64, 128, 64
K, DEG, G, LO, HI = 5, 3, 3, -1.0, 1.0
MACRO = 1024  # batch rows per device macro-iteration

_nc_cache = {}


def _basis_table():
    knots = np.linspace(LO - DEG * 0.1, HI + DEG * 0.1, K + DEG + 1)
    grid = np.linspace(LO, HI, G)
    bv = np.zeros((G, K), dtype=np.float32)
    for i in range(K):
        center = (knots[i + DEG // 2] + knots[i + DEG // 2 + 1]) / 2.0
        width = (knots[i + DEG + 1] - knots[i]) / 2.0
        bv[:, i] = np.exp(-(((grid - center) / width) ** 2))
    bv = bv / (bv.sum(axis=1, keepdims=True) + 1e-6)
    return bv


def _prep_consts(cp0, bw0, sw0, imp0, cp1, bw1, sw1, imp1):
    f8 = np.float64
    bv = _basis_table().astype(f8)
    d1, d2 = bv[1] - bv[0], bv[2] - bv[1]

    def fold(cp, bw, sw, imp):
        T = imp.astype(f8)[:, :, None] * sw.astype(f8)[:, :, None] * cp.astype(f8)
        Wb = imp.astype(f8) * bw.astype(f8)
        return Wb, T @ d1, T @ d2, (T @ bv[1]).sum(axis=0)

    Wb0, P10, P20, b1 = fold(cp0, bw0, sw0, imp0)
    Wb1, P11, P21, b2 = fold(cp1, bw1, sw1, imp1)
    bias2_eff = b2 + b1 @ P21

    w1 = np.stack([Wb0, P10, P20 - P10], axis=0)  # [3, 64, 128] lhsT chunks
    w1 = np.concatenate([w1, w1], axis=1)  # duplicate rows for partitions 64-127
    w1 = np.ascontiguousarray(w1.transpose(1, 0, 2)).reshape(128, 384)
    w2 = np.stack([Wb1, P11, P21 - P11], axis=0)  # [3, 128, 64] rhs chunks
    w2 = np.ascontiguousarray(w2.transpose(1, 0, 2)).reshape(128, 192)

    return {
        "wpk": np.concatenate([w1, w2], axis=1).astype(BF),  # [128, 576]
        "spk": np.stack(
            [b1, -1.0 - b1, 1.0 - b1, -b1, np.zeros_like(b1)], axis=1
        ).astype(np.float32),  # [128, 5] = b1|s1|s2|nb1|zero
        "b2row": np.tile(bias2_eff, 8).astype(BF).reshape(1, 512),
        "identc": np.eye(128, dtype=np.float32).astype(BF),
        "onesc": np.ones((1, 128), dtype=np.float32).astype(BF),
    }


def _build(rows):
    assert rows % MACRO == 0
    nc = bacc.Bacc(
        "TRN2",
        target_bir_lowering=False,
        debug=False,
        enable_asserts=False,
        num_devices=N_CORES,
    )
    xd = nc.dram_tensor("x", [rows, D0], F32, kind="ExternalInput")
    wpkd = nc.dram_tensor("wpk", [128, 576], BF16, kind="ExternalInput")
    spkd = nc.dram_tensor("spk", [128, 5], F32, kind="ExternalInput")
    b2d = nc.dram_tensor("b2row", [1, 512], BF16, kind="ExternalInput")
    identd = nc.dram_tensor("identc", [128, 128], BF16, kind="ExternalInput")
    onesd = nc.dram_tensor("onesc", [1, 128], BF16, kind="ExternalInput")
    outd = nc.dram_tensor("out", [rows, D2], F32, kind="ExternalOutput")

    n_macro = rows // MACRO
    MAX, MIN = mybir.AluOpType.max, mybir.AluOpType.min
    SILU = mybir.ActivationFunctionType.Silu

    with tile.TileContext(nc) as tc, ExitStack() as ctx:
        consts = ctx.enter_context(tc.tile_pool(name="consts", bufs=1))
        xin = ctx.enter_context(tc.tile_pool(name="xin", bufs=4))
        f1 = ctx.enter_context(tc.tile_pool(name="f1", bufs=3))
        f2 = ctx.enter_context(tc.tile_pool(name="f2", bufs=3))
        osb = ctx.enter_context(tc.tile_pool(name="osb", bufs=3))
        ps_x = ctx.enter_context(tc.tile_pool(name="ps_x", bufs=2, space="PSUM"))
        ps_h = ctx.enter_context(tc.tile_pool(name="ps_h", bufs=2, space="PSUM"))
        ps_o = ctx.enter_context(tc.tile_pool(name="ps_o", bufs=2, space="PSUM"))

        # ident/ones come in as DRAM constants: no gpsimd iota / DVE memset in
        # the preamble, so the input DMAs and PE warmup start sooner.
        ident = consts.tile([128, 128], BF16)
        nc.sync.dma_start(ident, identd.ap())
        ones = consts.tile([1, 128], BF16)
        nc.sync.dma_start(ones, onesd.ap())
        wpk = consts.tile([128, 576], BF16)
        nc.sync.dma_start(wpk, wpkd.ap())
        spk = consts.tile([128, 5], F32)
        nc.sync.dma_start(spk, spkd.ap())
        b2r = consts.tile([1, 512], BF16)
        nc.sync.dma_start(b2r, b2d.ap())
        b1, s1, s2, nb1, zc = (spk[:, i : i + 1] for i in range(5))
        w1c = [wpk[:, c * 128 : (c + 1) * 128] for c in range(3)]
        w2c = [wpk[:, 384 + c * 64 : 384 + (c + 1) * 64] for c in range(3)]

        # Preload the ACT Silu table during the preamble so the first real
        # activation doesn't stall 1.3us on a lazy ACT_TABLE_LOAD.
        slwarm = consts.tile([1, 128], BF16)
        nc.scalar.activation(slwarm, ones, SILU)

        # PE pre-warm: a few dummy matmuls while the input DMA lands (so the
        # HAM clock gate starts opening), the rest issued AFTER the first
        # transpose so they fill the PE's maps1(0) wait window and the clock
        # reaches full speed before the first real L1 matmuls.
        warm = ps_o.tile([128, 8, 64], F32, tag="po")
        for _ in range(4):
            nc.tensor.matmul(warm[:, 0:2], ident, ident, start=True, stop=True)

        def in_dma(m):
            # x[base + 8p + j, f] -> xt[p, j, f]: 2KB contiguous per
            # partition, f32 -> bf16 cast on the SWDGE queue.
            xt = xin.tile([128, 8, 64], BF16, tag="xt")
            src = bass.AP(xd, m * MACRO * 64, [[512, 128], [1, 512]])
            nc.gpsimd.dma_start(xt, src)
            return xt

        def transpose(xt):
            # PE transpose: px[:, q] partitions 0-63 = feats of rows 8p+2q
            # (A-stream), 64-127 = rows 8p+2q+1 (B-stream); free dim = p.
            px = ps_x.tile([128, 4, 128], BF16, tag="px")
            for q in range(4):
                nc.tensor.transpose(px[:, q], xt[:, 2 * q : 2 * q + 2], ident)
            return px

        def maps1(px):
            # L1 feature maps: u1/c1 chained on DVE, silu on ACT
            u1 = f1.tile([128, 4, 128], BF16, tag="u1")
            nc.vector.tensor_scalar(u1, px, -1.0, 1.0, op0=MAX, op1=MIN)
            sl1 = f1.tile([128, 4, 128], BF16, tag="sl1")
            nc.scalar.activation(sl1, px, SILU)
            c1 = f1.tile([128, 4, 128], BF16, tag="c1")
            nc.vector.tensor_scalar_max(c1, u1, zc)
            return u1, sl1, c1

        def l1_mm(mp):
            # L1: two concurrent 64-contraction row-group streams (A=even j
            # on partitions 0-63, B=odd j on 64-127) into one 2-bank tile so
            # sl2 can read all of h with a single merged instruction; chunk
            # order = feature readiness order (u1 -> sl1 -> c1)
            u1, sl1, c1 = mp
            h2 = ps_h.tile([128, 2, 512], F32, tag="h2")
            hA, hB = h2[:, 0], h2[:, 1]
            for i, (c, ft) in enumerate([(1, u1), (0, sl1), (2, c1)]):
                nc.tensor.matmul(hA, w1c[c][0:64], ft[0:64], start=(i == 0), stop=(i == 2))
                nc.tensor.matmul(hB, w1c[c][64:128], ft[64:128], start=(i == 0), stop=(i == 2))
            return h2

        def maps2_first(h2):
            # First half of the L2 feature maps.  The merged sl2 reads BOTH h
            # banks, so Tile serializes it after u2-B (bank B) and before
            # u2-A (bank A): a 2.7us cross-engine chain that would bound the
            # period.  maps1(m+1) is issued between the halves so the DVE
            # fills the sl2 window with u1/c1 instead of idling.
            sl2 = f2.tile([128, 1024], BF16, tag="sl2")
            u2 = f2.tile([128, 1024], BF16, tag="u2")
            nc.vector.tensor_scalar(u2[:, 512:1024], h2[:, 1], s1, s2, op0=MAX, op1=MIN)
            nc.scalar.activation(sl2, h2, SILU, bias=b1)
            return sl2, u2

        def maps2_second(m2, h2):
            sl2, u2 = m2
            c2 = f2.tile([128, 1024], BF16, tag="c2")
            nc.vector.tensor_scalar(u2[:, 0:512], h2[:, 0], s1, s2, op0=MAX, op1=MIN)
            nc.vector.tensor_scalar_max(c2, u2, nb1)
            return sl2, u2, c2

        def l2_mm(sl2, u2, c2):
            # L2 bias init via K=1 ones-matmul (sets has_written on the whole
            # bank so the 24 block matmuls accumulate with start=False).
            po = ps_o.tile([128, 8, 64], F32, tag="po")
            nc.tensor.matmul(po, ones, b2r, start=True, stop=False)
            # Blocks ordered by map readiness: u2's B-half lands first (it
            # is issued before the merged sl2), then sl2, then u2-A, then c2.
            plan = [
                (1, u2, (1, 3, 5, 7)),
                (0, sl2, (0, 2, 4, 6, 1, 3, 5, 7)),
                (1, u2, (0, 2, 4, 6)),
                (2, c2, (0, 1, 2, 3, 4, 5, 6, 7)),
            ]
            for ci, (c, ft, order) in enumerate(plan):
                for gi, g in enumerate(order):
                    off = (g % 2) * 512 + (g // 2) * 128
                    nc.tensor.matmul(
                        po[:, g],
                        ft[:, off : off + 128],
                        w2c[c],
                        start=False,
                        stop=(ci == 3 and gi == 7),
                    )
            return po

        def out_store(m, po):
            # po[p, g, f] = out row 8p+g -> contiguous 2KB per partition
            ot = osb.tile([128, 8, 64], F32, tag="ot")
            nc.scalar.copy(ot, po)
            dst = bass.AP(outd, m * MACRO * 64, [[512, 128], [1, 512]])
            nc.sync.dma_start(dst, ot)

        # 3-stage modulo software pipeline.  Issue order is chosen so each
        # in-order engine queue never head-blocks: PE = [tp(m+1), bias(m),
        # L2(m), L1(m+1)], ACT = [sl2(m), sl1(m+1), ot(m)],
        # DVE = [u2B(m), u1(m+1), c1(m+1), u2A(m), c2(m)].
        xt0 = in_dma(0)
        xt_next = in_dma(1)
        px = transpose(xt0)
        for _ in range(16):
            nc.tensor.matmul(warm[:, 0:2], ident, ident, start=True, stop=True)
        h2 = l1_mm(maps1(px))

        for m in range(n_macro):
            m2 = maps2_first(h2)

            # next macro's transpose + L1 maps: the transposes slot ahead of
            # bias/L2 in the PE queue, and u1/c1 fill the DVE while the
            # merged sl2 holds both h banks
            last = m + 1 >= n_macro
            if not last:
                px = transpose(xt_next)
                if m + 2 < n_macro:
                    xt_next = in_dma(m + 2)
                mp = maps1(px)

            f2m = maps2_second(m2, h2)
            po = l2_mm(*f2m)

            # next macro's L1 matmuls, issued before ot(m) so the ACT queue
            # never idles waiting on this macro's L2 matmuls
            if not last:
                h2 = l1_mm(mp)

            out_store(m, po)

    nc.compile()
    return nc


def _get_nc(rows):
    if rows not in _nc_cache:
        _nc_cache[rows] = _build(rows)
    return _nc_cache[rows]


def kernel(x, cp0, bw0, sw0, imp0, cp1, bw1, sw1, imp1, _trace=False, _trace_kwargs=None):
    x = np.ascontiguousarray(np.asarray(x, dtype=np.float32))
    consts = _prep_consts(
        *[np.asarray(a, dtype=np.float32) for a in (cp0, bw0, sw0, imp0, cp1, bw1, sw1, imp1)]
    )
    rows = x.shape[0] // N_CORES
    nc = _get_nc(rows)
    in_maps = []
    for i in range(N_CORES):
        m = dict(consts)
        m["x"] = x[i * rows : (i + 1) * rows]
        in_maps.append(m)
    res = run_bass_kernel_spmd(
        nc, in_maps, list(range(N_CORES)), trace=_trace, **(_trace_kwargs or {})
    )
    out = np.concatenate([res.results[i]["out"] for i in range(N_CORES)], axis=0)
    if _trace:
        return out, res
    return out
